# revision 38
# baseline (speedup 1.0000x reference)
"""CASSI shear kernel for Trainium2 (Bass/Tile), 8-core SPMD.

Computes, for full inputs x (1, 1024, 1024, 31) and ca (1, 1024, 1024, 1):
    y1[m, n, l] = x[m, n, l] * ca[m, n]
    out[m, j]   = sum_{n+l=j} y1[m, n, l]       (j in [0, 1054))
returning (1, 1024, 1054, 1) float32.

Sharding: rows m across 8 cores (128 rows/core = one full SBUF partition
block). Per core, free dim holds the (n, l) plane contiguously (n-major,
matching HBM layout so DMA loads are fully contiguous per partition).

The kernel is HBM-bound: 16.25 MB of x per core per pass = ~45 us at the
358 GB/s per-core HBM limit. Everything else is sized to hide under that:

  - DMA (SWDGE): x is loaded in 8 chunks of 128 n-columns, cast f32->bf16
    in the DMA datapath (halves SBUF footprint, enables 2x DVE + full-rate
    PE). All 8 chunk tiles stay resident so DMA never stalls on compute.
  - Vector engine: y1 = x * cab in place, bf16 2x_1P mode (~2.2 us/chunk).
    cab is ca broadcast 31x along l, materialized once in the preamble by
    the (otherwise idle) scalar engine, per chunk slice so chunk 0 never
    waits on the whole build.
  - Tensor engine: the 31-way shear scatter-add as bf16 identity-weight
    matmuls accumulating into PSUM. One matmul covers ALL 31 l values over
    a 16-wide n window: out free dims [w=16, g=31] with psum column
    t = t0 + n + k (overlapping within the op is fine -- PSUM accumulation
    is in-memory per element), rhs free dims [w, g] = y1[t0-l0+n, l0+k],
    which is one FLAT CONTIGUOUS bf16 stream. Dim order matters: putting
    the stride-31 n dim innermost instead ran the PE ~4x slower (strided
    rhs fetch). Windows split at PSUM bank boundaries (per-l fallback at
    the two crossings). PSUM has_written is reset by marking the first
    matmul touching each bank start=True (clears the whole bank) -- no
    zero-weight reset matmuls needed.
  - Scalar engine: evacuates each PSUM bank to SBUF as soon as its last
    contribution lands (bank 0 halfway through, banks 1-2 at the end), so
    only the last chunk's compute + one small copy + store sit after the
    final DMA.

The benchmark loop (loop_iters=N) wraps the body in For_i, whose back
edge runs an InstAllEngineBarrier costing ~5-10 us; "full@uN" unrolls N
bodies per back-edge to amortize it (u16 is the sweet spot: u1/u2/u4/
u8/u16/u32 measured 58.3/53.5/51.2/50.1/50.0/50.0 us with the staged-
input timing harness in test.py; u32 starts IRAM-thrashing).

Measured steady-state decomposition at u16 (per iteration, per core):
load stream 46.5 us (16.25 MB = 350 GB/s, ~98% of the 358 GB/s
HBM-per-NC limit), + mul 0.3, + PE/evac 0.3, + output stores 2.9
(1.4 us of write bytes + ~1.5 us HBM read/write-turnaround tax).

Dead ends measured (clean instrument, all within noise or worse):
"@hw" HWDGE f32 loads + DVE fused cast-mul (dodges the DVE-2port/SWDGE
descriptor-starvation trap -- but that trap only costs ~0.3 us here),
"@2q" alternating loads on both HWDGE rings, "@dgN" grouped bigger
DMAs, "@bs" single batched store, "@ds" stores deferred to the block
edge, "@gs" stores via SWDGE, "@s" staggered-reset stages (much
slower -- stage barriers break chunk pipelining), "@h" PE branch
prefetch hint, "@nc" no-carry evacuation (extra PE pieces cost more
than the DVE carry merges they remove), "@ck256"/"@ck512" coarser
chunking (neutral on the pure DMA stream, worse end-to-end: +1.2 us
at ck256, +4 us at ck512 where 2 xp bufs stall the pipeline). The store turnaround tax is
proportional to write bytes and survives every relocation of the
stores, so ~49.3 us (load stream + store bytes + barrier/16) is the
practical floor; the kernel sits ~0.7 us above it.
"""

import sys

import numpy as np

if "/opt/trn_rl_repo" not in sys.path:
    sys.path.insert(0, "/opt/trn_rl_repo")

M, N, L = 1024, 1024, 31
ONC = N + L - 1  # 1054
NCORES = 8
R = M // NCORES  # 128 rows per core
CHUNK = 128
BANK = 512  # PSUM bank size in fp32 elements

_cached_nc = {}


def _shear_pieces(chunk, gmax=31, carry=False):
    """All shear matmuls as {chunk_idx: [(l0, g, t0, w, start, stop)]}.

    Each matmul handles a group of g l-values {l0..l0+g-1} over the
    chunk's n-window: out free dims [g, w] with psum column t = t0 + k + n
    (overlapping within the op is fine -- PSUM accumulation is in-memory
    per element), rhs free dims [g, w] reading y1[(t0 - l0) + n, l0 + k].

    Windows split so each piece stays inside one PSUM bank. start=True
    marks the first matmul touching each bank (clears has_written for the
    whole bank -> accumulator resets with zero extra instructions);
    stop=True marks the last, gating that bank's evacuation.
    """
    if isinstance(chunk, int):
        chunks = [(i * chunk, chunk) for i in range(N // chunk)]
    else:
        chunks = chunk
    pieces = []
    for i, (n0, cw) in enumerate(chunks):
        for l0 in range(0, L, gmax):
            g = min(gmax, L - l0)
            wmax = BANK // gmax  # keep out free size within one bank
            t0 = n0 + l0
            remaining = cw
            while remaining > 0:
                bank_end = (t0 // BANK + 1) * BANK
                w = min(remaining, wmax, bank_end - t0 - (g - 1))
                if w < 1:
                    if carry:
                        # group span straddles the bank boundary: route
                        # the whole straddle rectangle (all g l's, the
                        # n's whose span crosses) into the carry bank as
                        # ONE matmul; merged into the output during
                        # evacuation. Carry col = CARRY_OFF[be] + (out
                        # col - (be - (g-1) - 1))... here simply
                        # cdst = carry base + (t0 - (be - 30)).
                        # out free size (wc*g) must stay <= 512 (fp32
                        # PSUM ISA limit), so split the straddle region
                        # into wmax-wide windows like the main pieces
                        wc = min(remaining, bank_end - t0, wmax)
                        cbase = 0 if bank_end == BANK else 64
                        cdst = cbase + (t0 - (bank_end - 30))
                        assert 0 <= cdst and cdst + wc - 1 + (g - 1) < cbase + 60
                        pieces.append(
                            [i, l0, g, t0, wc, False, False, cdst]
                        )
                        t0 += wc
                        remaining -= wc
                        continue
                    # no-carry fallback: emit the rest of this window
                    # per-l (small free dims)
                    for k in range(g):
                        aa, rem2 = t0 + k, remaining
                        while rem2 > 0:
                            be = (aa // BANK + 1) * BANK
                            w2 = min(rem2, be - aa)
                            pieces.append(
                                [i, l0 + k, 1, aa, w2, False, False, None]
                            )
                            aa += w2
                            rem2 -= w2
                    break
                pieces.append([i, l0, g, t0, w, False, False, None])
                t0 += w
                remaining -= w
    first_by_bank, last_by_bank = {}, {}
    for idx, (_, _, g, t0, w, _, _, cdst) in enumerate(pieces):
        # a piece touches banks floor(t0/BANK) .. floor((t0+g-1+w-1)/BANK);
        # by construction it stays in one bank ("carry" = the carry bank)
        b = "carry" if cdst is not None else t0 // BANK
        first_by_bank.setdefault(b, idx)
        last_by_bank[b] = idx
    for idx in first_by_bank.values():
        pieces[idx][5] = True
    for idx in last_by_bank.values():
        pieces[idx][6] = True
    by_chunk = {}
    for i, l0, g, t0, w, start, stop, cdst in pieces:
        by_chunk.setdefault(i, []).append((l0, g, t0, w, start, stop, cdst))
    # which banks see their final write in chunk i (drives evacuation)
    done_banks = {}
    for b, idx in last_by_bank.items():
        if b != "carry":
            done_banks.setdefault(pieces[idx][0], []).append(b)
    return by_chunk, done_banks


def _build_nc(loop_iters=None, variant="full"):
    """Build the per-core Bass program. loop_iters wraps the body in an
    on-device For_i repeating the computation (for benchmarking); None
    runs it once. variant: "full", or "+"-joined flags out of
    {dma, mul, pe} with optional "@u<unroll>" suffix."""
    key = (loop_iters, variant)
    if key in _cached_nc:
        return _cached_nc[key]

    import concourse.bass as bass
    import concourse.mybir as mybir
    from concourse import bacc
    from concourse.tile import TileContext

    f32 = mybir.dt.float32
    bf16 = mybir.dt.bfloat16
    nc = bacc.Bacc("TRN2")

    xin = nc.dram_tensor("x", (R, N * L), f32, kind="ExternalInput")
    cain = nc.dram_tensor("ca", (R, N), f32, kind="ExternalInput")
    identin = nc.dram_tensor("ident", (R, R), f32, kind="ExternalInput")
    outd = nc.dram_tensor("out", (R, ONC), f32, kind="ExternalOutput")

    toks = variant.split("@")
    vspec = toks[0]
    unroll, gmax, staggered, hint, tapered, use_carry = 1, 31, False, False, False, True
    use_hw, dma_group, no_store, two_q = False, 1, False, False
    batch_store, gp_store, defer_store, xp_bufs = False, False, False, None
    chunk_w = CHUNK
    for t in toks[1:]:
        if t == "s":
            staggered = True
        elif t == "h":
            hint = True
        elif t == "t":
            tapered = True
        elif t == "c":
            use_carry = True
        elif t == "nc":
            use_carry = False
        elif t == "hw":
            use_hw = True
        elif t == "2q":
            two_q = True
        elif t == "ns":
            no_store = True
        elif t == "bs":
            batch_store = True
        elif t == "gs":
            gp_store = True
        elif t == "ds":
            defer_store = True
        elif t.startswith("xb"):
            xp_bufs = int(t[2:])
        elif t.startswith("ck"):
            chunk_w = int(t[2:])
        elif t.startswith("dg"):
            dma_group = int(t[2:])
        elif t.startswith("u"):
            unroll = int(t[1:])
        elif t.startswith("g"):
            gmax = int(t[1:])
    # the carry-merge geometry holds for chunk widths where the two bank
    # straddles (cols 482.., 994..) land in chunks nchunks//2-1 and
    # nchunks-1 with the same carry-bank offsets: 128/256/512
    if tapered or gmax != 31 or chunk_w not in (128, 256, 512):
        use_carry = False
    if vspec == "full":
        flags = {"dma", "mul", "pe"}
    else:
        flags = set(vspec.split("+"))
    if tapered:
        # split the last 128-col chunk in two: halves the serial tail
        # (mul + shear of the final chunk) behind the last DMA
        chunks = [(i * CHUNK, CHUNK) for i in range(N // CHUNK - 1)]
        h = CHUNK // 2
        chunks += [(N - CHUNK, h), (N - h, h)]
    else:
        chunks = [(i * chunk_w, chunk_w) for i in range(N // chunk_w)]
    nchunks = len(chunks)
    by_chunk, done_banks = _shear_pieces(chunks, gmax, carry=use_carry)
    if use_carry:
        # hardcoded merge geometry below assumes this piece layout
        assert not tapered and gmax == 31
        carry_ia = nchunks // 2 - 1  # chunk containing the col-482 straddle
        carry_ib = nchunks - 1  # chunk containing the col-994 straddle
        assert sorted(done_banks.get(carry_ia, [])) == [0]
        assert sorted(done_banks.get(carry_ib, [])) == [1]

    # hw mode: x loads as f32 on HWDGE (sync engine). SWDGE cast-DMAs are
    # starved whenever DVE runs a two-read-port op (the tensor_tensor mul
    # holds the DVE/GpSimd shared SBUF port pair, blocking Q7 descriptor
    # generation); HWDGE has no SBUF descriptor rings, so it is immune.
    # The f32->bf16 cast folds into the multiply (DVE converts on write),
    # and ca is read via a stride-0 broadcast AP, so the cab materialization
    # disappears too. Stores ride the other HWDGE ring (scalar/ACT).
    if use_hw:
        assert nchunks % dma_group == 0
        store_eng = nc.scalar
    else:
        dma_group = 1
        store_eng = nc.sync
    if gp_store:
        store_eng = nc.gpsimd

    acc_bufs = (unroll + 1) if defer_store else 2
    with TileContext(nc) as tc:
        with (
            tc.tile_pool(name="xp", bufs=xp_bufs or ((4 if dma_group <= 2 else 2) if use_hw else nchunks)) as xp,
            tc.tile_pool(name="yp", bufs=4) as yp,
            tc.tile_pool(name="cp", bufs=1) as cp,
            tc.tile_pool(name="accp", bufs=acc_bufs) as accp,
            tc.tile_pool(name="pp", bufs=2, space="PSUM") as pp,
        ):
            ca_t = cp.tile([R, N], f32)
            nc.sync.dma_start(out=ca_t[:], in_=cain[:])
            # bf16 identity for the shear matmuls (cast during DMA)
            idb = cp.tile([R, R], bf16, tag="idb")
            nc.gpsimd.dma_start(out=idb[:], in_=identin[:])

            cab = None
            if not use_hw:
                # cab[m, n*L + l] = ca[m, n] as bf16: built once, per chunk
                # slice, on the scalar engine (idle during the main loop)
                cab = cp.tile([R, N * L], bf16, tag="cab")
                cab3 = cab[:].rearrange("p (n l) -> p n l", l=L)
                for n0, cw in chunks:
                    src = (
                        ca_t[:, n0 : n0 + cw]
                        .unsqueeze(2)
                        .broadcast_to([R, cw, L])
                    )
                    nc.scalar.copy(cab3[:, n0 : n0 + cw], src)

            def body(marks=(), pending=None):
                def store(dst, src):
                    if no_store:
                        return
                    if pending is None:
                        store_eng.dma_start(out=dst, in_=src)
                    else:
                        pending.append((dst, src))

                pacc = pp.tile([R, ONC], f32, tag="pacc")
                pc = None
                if use_carry:
                    pc = pp.tile([R, 128], f32, tag="carry")
                xts = {}
                for i, (n0, cw) in enumerate(chunks):
                    if i in marks:
                        tc.stage_boundary()
                    if use_hw:
                        if i % dma_group == 0:
                            gw = sum(c for _, c in chunks[i : i + dma_group])
                            gt = xp.tile([R, gw * L], f32, tag="xgrp")
                            if "dma" in flags:
                                # alternate loads across the two HWDGE
                                # rings (qSP / qAct) when two_q
                                ldq = (
                                    nc.scalar
                                    if two_q and (i // dma_group) % 2
                                    else nc.sync
                                )
                                ldq.dma_start(
                                    out=gt[:],
                                    in_=xin[:, n0 * L : n0 * L + gw * L],
                                )
                            for j, (m0, mw) in enumerate(
                                chunks[i : i + dma_group]
                            ):
                                off = (m0 - n0) * L
                                xts[i + j] = gt[:, off : off + mw * L]
                        yv = None
                        if "mul" in flags:
                            yt = yp.tile([R, cw * L], bf16, tag="ychunk")
                            src3 = xts[i].rearrange(
                                "p (n l) -> p n l", l=L
                            )
                            cb3 = (
                                ca_t[:, n0 : n0 + cw]
                                .unsqueeze(2)
                                .broadcast_to([R, cw, L])
                            )
                            y3 = yt[:].rearrange("p (n l) -> p n l", l=L)
                            nc.vector.tensor_tensor(
                                y3, src3, cb3, mybir.AluOpType.mult
                            )
                            yv = yt[:]
                    else:
                        xt = xp.tile([R, cw * L], bf16, tag="xchunk")
                        if "dma" in flags:
                            # f32 -> bf16 cast in the DMA datapath (SWDGE)
                            nc.gpsimd.dma_start(
                                out=xt[:], in_=xin[:, n0 * L : (n0 + cw) * L]
                            )
                        yv = xt[:]
                        if "mul" in flags:
                            # in-place broadcast multiply, bf16 2x_1P (both
                            # operands contiguous step-1 bf16)
                            nc.vector.tensor_tensor(
                                yv,
                                yv,
                                cab[:, n0 * L : (n0 + cw) * L],
                                mybir.AluOpType.mult,
                            )
                    if "pe" in flags and yv is not None:
                        part = [int(yv.ap[0][0]), int(yv.ap[0][1])]
                        for l0, g, t0, w, start, stop, cdst in by_chunk[i]:
                            # out col t = t0 + k + n (overlap inside the
                            # op is fine; PSUM accumulation is in-memory);
                            # rhs elem (n,k) = y1[(t0-l0-n0)+n, l0+k].
                            # Dim order: n outer, l-group inner -- the
                            # innermost run is contiguous in SBUF (the PE
                            # rhs fetch rate collapses on strided inner
                            # reads; at g=31 the whole stream is flat)
                            rhs = bass.AP(
                                yv.tensor,
                                yv.offset + (t0 - l0 - n0) * L + l0,
                                [part, [L, w], [1, g]],
                            )
                            if cdst is None:
                                pv = pacc[:, t0 : t0 + (g - 1) + w]
                            else:
                                pv = pc[:, cdst : cdst + (g - 1) + w]
                            pp0 = [int(pv.ap[0][0]), int(pv.ap[0][1])]
                            dst = bass.AP(
                                pv.tensor, pv.offset, [pp0, [1, w], [1, g]]
                            )
                            nc.tensor.matmul(
                                dst,
                                idb[:],
                                rhs,
                                start=start,
                                stop=stop,
                                skip_group_check=True,
                            )
                        if use_carry:
                            # explicit evacuation with carry merges
                            # (geometry asserted above)
                            add = mybir.AluOpType.add
                            if i == carry_ia:
                                if batch_store:
                                    acc = accp.tile([R, ONC], f32,
                                                    tag="accall")
                                    body.acc = acc
                                    at0 = acc[:, 0:BANK]
                                else:
                                    at0t = accp.tile([R, BANK], f32,
                                                     tag="acc0")
                                    at0 = at0t[:]
                                nc.scalar.copy(at0, pacc[:, 0:BANK])
                                nc.vector.tensor_tensor(
                                    at0[:, 482:512], at0[:, 482:512],
                                    pc[:, 0:30], add,
                                )
                                if not batch_store:
                                    store(outd[:, 0:BANK], at0)
                            elif i == carry_ib:
                                if batch_store:
                                    at1 = body.acc[:, BANK:ONC]
                                else:
                                    at1t = accp.tile([R, ONC - BANK], f32,
                                                     tag="acc1")
                                    at1 = at1t[:]
                                nc.scalar.copy(
                                    at1[:, 0:512], pacc[:, 512:1024]
                                )
                                nc.vector.tensor_tensor(
                                    at1[:, 0:30], at1[:, 0:30],
                                    pc[:, 30:60], add,
                                )
                                nc.vector.tensor_tensor(
                                    at1[:, 482:512], at1[:, 482:512],
                                    pc[:, 64:94], add,
                                )
                                nc.vector.tensor_copy(
                                    at1[:, 512:542], pc[:, 94:124]
                                )
                                if batch_store:
                                    store(outd[:, 0:ONC], body.acc[:])
                                else:
                                    store(outd[:, BANK:ONC], at1)
                            continue
                        # evacuate any PSUM banks whose last contribution
                        # just landed (adjacent banks coalesced); store
                        # them right away
                        bs = sorted(done_banks.get(i, []))
                        while bs:
                            b0 = b1 = bs.pop(0)
                            while bs and bs[0] == b1 + 1:
                                b1 = bs.pop(0)
                            a0 = b0 * BANK
                            a1 = min((b1 + 1) * BANK, ONC)
                            at = accp.tile([R, a1 - a0], f32, tag=f"acc{b0}")
                            nc.scalar.copy(at[:], pacc[:, a0:a1])
                            store(outd[:, a0:a1], at[:])

            if loop_iters is None:
                body()
            else:
                u = max(u for u in (unroll, 1) if loop_iters % u == 0)
                hints = (mybir.EngineType.PE,) if hint else ()
                stag = staggered and (u * nchunks) % 4 == 0
                # staggered_reset needs exactly 4 stages per loop body;
                # spread them evenly over the unrolled copies' chunks
                span = (u * nchunks) // 4
                with tc.For_i(
                    0, loop_iters // u, 1,
                    hint_engines=hints, staggered_reset=stag,
                ):
                    pend = [] if defer_store else None
                    for j in range(u):
                        if stag:
                            marks = {
                                (s * span) - j * nchunks
                                for s in range(1, 4)
                                if 0 <= (s * span) - j * nchunks < nchunks
                            }
                        else:
                            marks = ()
                        body(marks, pend)
                    if pend:
                        # flush all deferred output stores at the block
                        # edge: the write bytes drain while the loop
                        # barrier has the load stream quiesced instead of
                        # displacing read bandwidth mid-iteration
                        for dst, src in pend:
                            store_eng.dma_start(out=dst, in_=src)

    nc.finalize()
    _cached_nc[key] = nc
    return nc


_IDENT = None


def _run(x_slab, ca_slab, loop_iters=None, variant="full", **run_kwargs):
    """x_slab (M, N*L) f32, ca_slab (M, N) f32 -> (M, ONC) f32."""
    from concourse.bass_utils import run_bass_kernel_spmd

    global _IDENT
    if _IDENT is None:
        _IDENT = np.eye(R, dtype=np.float32)

    nc = _build_nc(loop_iters, variant)
    in_maps = []
    for c in range(NCORES):
        in_maps.append(
            {
                "x": np.ascontiguousarray(x_slab[c * R : (c + 1) * R]),
                "ca": np.ascontiguousarray(ca_slab[c * R : (c + 1) * R]),
                "ident": _IDENT,
            }
        )
    res = run_bass_kernel_spmd(nc, in_maps, core_ids=list(range(NCORES)), **run_kwargs)
    out = np.concatenate(
        [np.asarray(res.results[c]["out"]) for c in range(NCORES)], axis=0
    )
    return out, res


def kernel(x, ca):
    x = np.ascontiguousarray(np.asarray(x, dtype=np.float32).reshape(M, N * L))
    ca = np.ascontiguousarray(np.asarray(ca, dtype=np.float32).reshape(M, N))
    out, _ = _run(x, ca)
    return out.reshape(1, M, ONC, 1)



# revision 40
# speedup vs baseline: 1.0160x; 1.0160x over previous
"""CASSI shear kernel for Trainium2 (Bass/Tile), 8-core SPMD.

Computes, for full inputs x (1, 1024, 1024, 31) and ca (1, 1024, 1024, 1):
    y1[m, n, l] = x[m, n, l] * ca[m, n]
    out[m, j]   = sum_{n+l=j} y1[m, n, l]       (j in [0, 1054))
returning (1, 1024, 1054, 1) float32.

Sharding: rows m across 8 cores (128 rows/core = one full SBUF partition
block). Per core, free dim holds the (n, l) plane contiguously (n-major,
matching HBM layout so DMA loads are fully contiguous per partition).

The kernel is HBM-bound: 16.25 MB of x per core per pass = ~45 us at the
358 GB/s per-core HBM limit. Everything else is sized to hide under that:

  - DMA (SWDGE): x is loaded in 8 chunks of 128 n-columns, cast f32->bf16
    in the DMA datapath (halves SBUF footprint, enables 2x DVE + full-rate
    PE). All 8 chunk tiles stay resident so DMA never stalls on compute.
  - Vector engine: y1 = x * cab in place, bf16 2x_1P mode (~2.2 us/chunk).
    cab is ca broadcast 31x along l, materialized once in the preamble by
    the (otherwise idle) scalar engine, per chunk slice so chunk 0 never
    waits on the whole build.
  - Tensor engine: the 31-way shear scatter-add as bf16 identity-weight
    matmuls accumulating into PSUM. One matmul covers ALL 31 l values over
    a 16-wide n window: out free dims [w=16, g=31] with psum column
    t = t0 + n + k (overlapping within the op is fine -- PSUM accumulation
    is in-memory per element), rhs free dims [w, g] = y1[t0-l0+n, l0+k],
    which is one FLAT CONTIGUOUS bf16 stream. Dim order matters: putting
    the stride-31 n dim innermost instead ran the PE ~4x slower (strided
    rhs fetch). Windows split at PSUM bank boundaries (per-l fallback at
    the two crossings). PSUM has_written is reset by marking the first
    matmul touching each bank start=True (clears the whole bank) -- no
    zero-weight reset matmuls needed.
  - Scalar engine: evacuates each PSUM bank to SBUF as soon as its last
    contribution lands (bank 0 halfway through, banks 1-2 at the end), so
    only the last chunk's compute + one small copy + store sit after the
    final DMA.

The benchmark loop (loop_iters=N) wraps the body in For_i, whose back
edge runs an InstAllEngineBarrier costing ~5-10 us; "full@uN" unrolls N
bodies per back-edge to amortize it (u1/u2/u4/u8/u16/u32 measured
58.3/53.5/51.2/50.1/50.0/50.0 us with the staged-input timing harness
in test.py; u32 starts IRAM-thrashing; u20 edges out u16 by ~0.25 us).
"@bs" batches the two per-body output stores into one full-row
1054-col DMA -- better HBM write locality, another ~0.25 us. The
benched champion is full@bs@u20 at ~49.8 us.

Measured steady-state decomposition at u16 (per iteration, per core):
load stream 46.5 us (16.25 MB = 350 GB/s, ~98% of the 358 GB/s
HBM-per-NC limit), + mul 0.3, + PE/evac 0.3, + output stores 2.9
(1.4 us of write bytes + ~1.5 us HBM read/write-turnaround tax).

Dead ends measured (clean instrument, all within noise or worse):
"@hw" HWDGE f32 loads + DVE fused cast-mul (dodges the DVE-2port/SWDGE
descriptor-starvation trap -- but that trap only costs ~0.3 us here),
"@2q" alternating loads on both HWDGE rings, "@dgN" grouped bigger
DMAs, "@bs" single batched store, "@ds" stores deferred to the block
edge, "@gs" stores via SWDGE, "@s" staggered-reset stages (much
slower -- stage barriers break chunk pipelining), "@h" PE branch
prefetch hint, "@nc" no-carry evacuation (extra PE pieces cost more
than the DVE carry merges they remove), "@ck256"/"@ck512" coarser
chunking (neutral on the pure DMA stream, worse end-to-end: +1.2 us
at ck256, +4 us at ck512 where 2 xp bufs stall the pipeline). Only
"@bs" (store batching) beat the turnaround tax, and only by ~0.25 us
of the ~1.5. The store turnaround tax is
proportional to write bytes and survives every relocation of the
stores, so ~49.3 us (load stream + store bytes + barrier/16) is the
practical floor; the kernel sits ~0.7 us above it.
"""

import sys

import numpy as np

if "/opt/trn_rl_repo" not in sys.path:
    sys.path.insert(0, "/opt/trn_rl_repo")

M, N, L = 1024, 1024, 31
ONC = N + L - 1  # 1054
NCORES = 8
R = M // NCORES  # 128 rows per core
CHUNK = 128
BANK = 512  # PSUM bank size in fp32 elements

_cached_nc = {}


def _shear_pieces(chunk, gmax=31, carry=False):
    """All shear matmuls as {chunk_idx: [(l0, g, t0, w, start, stop)]}.

    Each matmul handles a group of g l-values {l0..l0+g-1} over the
    chunk's n-window: out free dims [g, w] with psum column t = t0 + k + n
    (overlapping within the op is fine -- PSUM accumulation is in-memory
    per element), rhs free dims [g, w] reading y1[(t0 - l0) + n, l0 + k].

    Windows split so each piece stays inside one PSUM bank. start=True
    marks the first matmul touching each bank (clears has_written for the
    whole bank -> accumulator resets with zero extra instructions);
    stop=True marks the last, gating that bank's evacuation.
    """
    if isinstance(chunk, int):
        chunks = [(i * chunk, chunk) for i in range(N // chunk)]
    else:
        chunks = chunk
    pieces = []
    for i, (n0, cw) in enumerate(chunks):
        for l0 in range(0, L, gmax):
            g = min(gmax, L - l0)
            wmax = BANK // gmax  # keep out free size within one bank
            t0 = n0 + l0
            remaining = cw
            while remaining > 0:
                bank_end = (t0 // BANK + 1) * BANK
                w = min(remaining, wmax, bank_end - t0 - (g - 1))
                if w < 1:
                    if carry:
                        # group span straddles the bank boundary: route
                        # the whole straddle rectangle (all g l's, the
                        # n's whose span crosses) into the carry bank as
                        # ONE matmul; merged into the output during
                        # evacuation. Carry col = CARRY_OFF[be] + (out
                        # col - (be - (g-1) - 1))... here simply
                        # cdst = carry base + (t0 - (be - 30)).
                        # out free size (wc*g) must stay <= 512 (fp32
                        # PSUM ISA limit), so split the straddle region
                        # into wmax-wide windows like the main pieces
                        wc = min(remaining, bank_end - t0, wmax)
                        cbase = 0 if bank_end == BANK else 64
                        cdst = cbase + (t0 - (bank_end - 30))
                        assert 0 <= cdst and cdst + wc - 1 + (g - 1) < cbase + 60
                        pieces.append(
                            [i, l0, g, t0, wc, False, False, cdst]
                        )
                        t0 += wc
                        remaining -= wc
                        continue
                    # no-carry fallback: emit the rest of this window
                    # per-l (small free dims)
                    for k in range(g):
                        aa, rem2 = t0 + k, remaining
                        while rem2 > 0:
                            be = (aa // BANK + 1) * BANK
                            w2 = min(rem2, be - aa)
                            pieces.append(
                                [i, l0 + k, 1, aa, w2, False, False, None]
                            )
                            aa += w2
                            rem2 -= w2
                    break
                pieces.append([i, l0, g, t0, w, False, False, None])
                t0 += w
                remaining -= w
    first_by_bank, last_by_bank = {}, {}
    for idx, (_, _, g, t0, w, _, _, cdst) in enumerate(pieces):
        # a piece touches banks floor(t0/BANK) .. floor((t0+g-1+w-1)/BANK);
        # by construction it stays in one bank ("carry" = the carry bank)
        b = "carry" if cdst is not None else t0 // BANK
        first_by_bank.setdefault(b, idx)
        last_by_bank[b] = idx
    for idx in first_by_bank.values():
        pieces[idx][5] = True
    for idx in last_by_bank.values():
        pieces[idx][6] = True
    by_chunk = {}
    for i, l0, g, t0, w, start, stop, cdst in pieces:
        by_chunk.setdefault(i, []).append((l0, g, t0, w, start, stop, cdst))
    # which banks see their final write in chunk i (drives evacuation)
    done_banks = {}
    for b, idx in last_by_bank.items():
        if b != "carry":
            done_banks.setdefault(pieces[idx][0], []).append(b)
    return by_chunk, done_banks


def _build_nc(loop_iters=None, variant="full"):
    """Build the per-core Bass program. loop_iters wraps the body in an
    on-device For_i repeating the computation (for benchmarking); None
    runs it once. variant: "full", or "+"-joined flags out of
    {dma, mul, pe} with optional "@u<unroll>" suffix."""
    key = (loop_iters, variant)
    if key in _cached_nc:
        return _cached_nc[key]

    import concourse.bass as bass
    import concourse.mybir as mybir
    from concourse import bacc
    from concourse.tile import TileContext

    f32 = mybir.dt.float32
    bf16 = mybir.dt.bfloat16
    nc = bacc.Bacc("TRN2")

    xin = nc.dram_tensor("x", (R, N * L), f32, kind="ExternalInput")
    cain = nc.dram_tensor("ca", (R, N), f32, kind="ExternalInput")
    identin = nc.dram_tensor("ident", (R, R), f32, kind="ExternalInput")
    outd = nc.dram_tensor("out", (R, ONC), f32, kind="ExternalOutput")

    toks = variant.split("@")
    vspec = toks[0]
    unroll, gmax, staggered, hint, tapered, use_carry = 1, 31, False, False, False, True
    use_hw, dma_group, no_store, two_q = False, 1, False, False
    batch_store, gp_store, defer_store, xp_bufs = False, False, False, None
    chunk_w = CHUNK
    for t in toks[1:]:
        if t == "s":
            staggered = True
        elif t == "h":
            hint = True
        elif t == "t":
            tapered = True
        elif t == "c":
            use_carry = True
        elif t == "nc":
            use_carry = False
        elif t == "hw":
            use_hw = True
        elif t == "2q":
            two_q = True
        elif t == "ns":
            no_store = True
        elif t == "bs":
            batch_store = True
        elif t == "gs":
            gp_store = True
        elif t == "ds":
            defer_store = True
        elif t.startswith("xb"):
            xp_bufs = int(t[2:])
        elif t.startswith("ck"):
            chunk_w = int(t[2:])
        elif t.startswith("dg"):
            dma_group = int(t[2:])
        elif t.startswith("u"):
            unroll = int(t[1:])
        elif t.startswith("g"):
            gmax = int(t[1:])
    # the carry-merge geometry holds for chunk widths where the two bank
    # straddles (cols 482.., 994..) land in chunks nchunks//2-1 and
    # nchunks-1 with the same carry-bank offsets: 128/256/512
    if tapered or gmax != 31 or chunk_w not in (128, 256, 512):
        use_carry = False
    if vspec == "full":
        flags = {"dma", "mul", "pe"}
    else:
        flags = set(vspec.split("+"))
    if tapered:
        # split the last 128-col chunk in two: halves the serial tail
        # (mul + shear of the final chunk) behind the last DMA
        chunks = [(i * CHUNK, CHUNK) for i in range(N // CHUNK - 1)]
        h = CHUNK // 2
        chunks += [(N - CHUNK, h), (N - h, h)]
    else:
        chunks = [(i * chunk_w, chunk_w) for i in range(N // chunk_w)]
    nchunks = len(chunks)
    by_chunk, done_banks = _shear_pieces(chunks, gmax, carry=use_carry)
    if use_carry:
        # hardcoded merge geometry below assumes this piece layout
        assert not tapered and gmax == 31
        carry_ia = nchunks // 2 - 1  # chunk containing the col-482 straddle
        carry_ib = nchunks - 1  # chunk containing the col-994 straddle
        assert sorted(done_banks.get(carry_ia, [])) == [0]
        assert sorted(done_banks.get(carry_ib, [])) == [1]

    # hw mode: x loads as f32 on HWDGE (sync engine). SWDGE cast-DMAs are
    # starved whenever DVE runs a two-read-port op (the tensor_tensor mul
    # holds the DVE/GpSimd shared SBUF port pair, blocking Q7 descriptor
    # generation); HWDGE has no SBUF descriptor rings, so it is immune.
    # The f32->bf16 cast folds into the multiply (DVE converts on write),
    # and ca is read via a stride-0 broadcast AP, so the cab materialization
    # disappears too. Stores ride the other HWDGE ring (scalar/ACT).
    if use_hw:
        assert nchunks % dma_group == 0
        store_eng = nc.scalar
    else:
        dma_group = 1
        store_eng = nc.sync
    if gp_store:
        store_eng = nc.gpsimd

    acc_bufs = (unroll + 1) if defer_store else 2
    with TileContext(nc) as tc:
        with (
            tc.tile_pool(name="xp", bufs=xp_bufs or ((4 if dma_group <= 2 else 2) if use_hw else nchunks)) as xp,
            tc.tile_pool(name="yp", bufs=4) as yp,
            tc.tile_pool(name="cp", bufs=1) as cp,
            tc.tile_pool(name="accp", bufs=acc_bufs) as accp,
            tc.tile_pool(name="pp", bufs=2, space="PSUM") as pp,
        ):
            ca_t = cp.tile([R, N], f32)
            nc.sync.dma_start(out=ca_t[:], in_=cain[:])
            # bf16 identity for the shear matmuls (cast during DMA)
            idb = cp.tile([R, R], bf16, tag="idb")
            nc.gpsimd.dma_start(out=idb[:], in_=identin[:])

            cab = None
            if not use_hw:
                # cab[m, n*L + l] = ca[m, n] as bf16: built once, per chunk
                # slice, on the scalar engine (idle during the main loop)
                cab = cp.tile([R, N * L], bf16, tag="cab")
                cab3 = cab[:].rearrange("p (n l) -> p n l", l=L)
                for n0, cw in chunks:
                    src = (
                        ca_t[:, n0 : n0 + cw]
                        .unsqueeze(2)
                        .broadcast_to([R, cw, L])
                    )
                    nc.scalar.copy(cab3[:, n0 : n0 + cw], src)

            def body(marks=(), pending=None):
                def store(dst, src):
                    if no_store:
                        return
                    if pending is None:
                        store_eng.dma_start(out=dst, in_=src)
                    else:
                        pending.append((dst, src))

                pacc = pp.tile([R, ONC], f32, tag="pacc")
                pc = None
                if use_carry:
                    pc = pp.tile([R, 128], f32, tag="carry")
                xts = {}
                for i, (n0, cw) in enumerate(chunks):
                    if i in marks:
                        tc.stage_boundary()
                    if use_hw:
                        if i % dma_group == 0:
                            gw = sum(c for _, c in chunks[i : i + dma_group])
                            gt = xp.tile([R, gw * L], f32, tag="xgrp")
                            if "dma" in flags:
                                # alternate loads across the two HWDGE
                                # rings (qSP / qAct) when two_q
                                ldq = (
                                    nc.scalar
                                    if two_q and (i // dma_group) % 2
                                    else nc.sync
                                )
                                ldq.dma_start(
                                    out=gt[:],
                                    in_=xin[:, n0 * L : n0 * L + gw * L],
                                )
                            for j, (m0, mw) in enumerate(
                                chunks[i : i + dma_group]
                            ):
                                off = (m0 - n0) * L
                                xts[i + j] = gt[:, off : off + mw * L]
                        yv = None
                        if "mul" in flags:
                            yt = yp.tile([R, cw * L], bf16, tag="ychunk")
                            src3 = xts[i].rearrange(
                                "p (n l) -> p n l", l=L
                            )
                            cb3 = (
                                ca_t[:, n0 : n0 + cw]
                                .unsqueeze(2)
                                .broadcast_to([R, cw, L])
                            )
                            y3 = yt[:].rearrange("p (n l) -> p n l", l=L)
                            nc.vector.tensor_tensor(
                                y3, src3, cb3, mybir.AluOpType.mult
                            )
                            yv = yt[:]
                    else:
                        xt = xp.tile([R, cw * L], bf16, tag="xchunk")
                        if "dma" in flags:
                            # f32 -> bf16 cast in the DMA datapath (SWDGE)
                            nc.gpsimd.dma_start(
                                out=xt[:], in_=xin[:, n0 * L : (n0 + cw) * L]
                            )
                        yv = xt[:]
                        if "mul" in flags:
                            # in-place broadcast multiply, bf16 2x_1P (both
                            # operands contiguous step-1 bf16)
                            nc.vector.tensor_tensor(
                                yv,
                                yv,
                                cab[:, n0 * L : (n0 + cw) * L],
                                mybir.AluOpType.mult,
                            )
                    if "pe" in flags and yv is not None:
                        part = [int(yv.ap[0][0]), int(yv.ap[0][1])]
                        for l0, g, t0, w, start, stop, cdst in by_chunk[i]:
                            # out col t = t0 + k + n (overlap inside the
                            # op is fine; PSUM accumulation is in-memory);
                            # rhs elem (n,k) = y1[(t0-l0-n0)+n, l0+k].
                            # Dim order: n outer, l-group inner -- the
                            # innermost run is contiguous in SBUF (the PE
                            # rhs fetch rate collapses on strided inner
                            # reads; at g=31 the whole stream is flat)
                            rhs = bass.AP(
                                yv.tensor,
                                yv.offset + (t0 - l0 - n0) * L + l0,
                                [part, [L, w], [1, g]],
                            )
                            if cdst is None:
                                pv = pacc[:, t0 : t0 + (g - 1) + w]
                            else:
                                pv = pc[:, cdst : cdst + (g - 1) + w]
                            pp0 = [int(pv.ap[0][0]), int(pv.ap[0][1])]
                            dst = bass.AP(
                                pv.tensor, pv.offset, [pp0, [1, w], [1, g]]
                            )
                            nc.tensor.matmul(
                                dst,
                                idb[:],
                                rhs,
                                start=start,
                                stop=stop,
                                skip_group_check=True,
                            )
                        if use_carry:
                            # explicit evacuation with carry merges
                            # (geometry asserted above)
                            add = mybir.AluOpType.add
                            if i == carry_ia:
                                if batch_store:
                                    acc = accp.tile([R, ONC], f32,
                                                    tag="accall")
                                    body.acc = acc
                                    at0 = acc[:, 0:BANK]
                                else:
                                    at0t = accp.tile([R, BANK], f32,
                                                     tag="acc0")
                                    at0 = at0t[:]
                                nc.scalar.copy(at0, pacc[:, 0:BANK])
                                nc.vector.tensor_tensor(
                                    at0[:, 482:512], at0[:, 482:512],
                                    pc[:, 0:30], add,
                                )
                                if not batch_store:
                                    store(outd[:, 0:BANK], at0)
                            elif i == carry_ib:
                                if batch_store:
                                    at1 = body.acc[:, BANK:ONC]
                                else:
                                    at1t = accp.tile([R, ONC - BANK], f32,
                                                     tag="acc1")
                                    at1 = at1t[:]
                                nc.scalar.copy(
                                    at1[:, 0:512], pacc[:, 512:1024]
                                )
                                nc.vector.tensor_tensor(
                                    at1[:, 0:30], at1[:, 0:30],
                                    pc[:, 30:60], add,
                                )
                                nc.vector.tensor_tensor(
                                    at1[:, 482:512], at1[:, 482:512],
                                    pc[:, 64:94], add,
                                )
                                nc.vector.tensor_copy(
                                    at1[:, 512:542], pc[:, 94:124]
                                )
                                if batch_store:
                                    store(outd[:, 0:ONC], body.acc[:])
                                else:
                                    store(outd[:, BANK:ONC], at1)
                            continue
                        # evacuate any PSUM banks whose last contribution
                        # just landed (adjacent banks coalesced); store
                        # them right away
                        bs = sorted(done_banks.get(i, []))
                        while bs:
                            b0 = b1 = bs.pop(0)
                            while bs and bs[0] == b1 + 1:
                                b1 = bs.pop(0)
                            a0 = b0 * BANK
                            a1 = min((b1 + 1) * BANK, ONC)
                            at = accp.tile([R, a1 - a0], f32, tag=f"acc{b0}")
                            nc.scalar.copy(at[:], pacc[:, a0:a1])
                            store(outd[:, a0:a1], at[:])

            if loop_iters is None:
                body()
            else:
                u = max(u for u in (unroll, 1) if loop_iters % u == 0)
                hints = (mybir.EngineType.PE,) if hint else ()
                stag = staggered and (u * nchunks) % 4 == 0
                # staggered_reset needs exactly 4 stages per loop body;
                # spread them evenly over the unrolled copies' chunks
                span = (u * nchunks) // 4
                with tc.For_i(
                    0, loop_iters // u, 1,
                    hint_engines=hints, staggered_reset=stag,
                ):
                    pend = [] if defer_store else None
                    for j in range(u):
                        if stag:
                            marks = {
                                (s * span) - j * nchunks
                                for s in range(1, 4)
                                if 0 <= (s * span) - j * nchunks < nchunks
                            }
                        else:
                            marks = ()
                        body(marks, pend)
                    if pend:
                        # flush all deferred output stores at the block
                        # edge: the write bytes drain while the loop
                        # barrier has the load stream quiesced instead of
                        # displacing read bandwidth mid-iteration
                        for dst, src in pend:
                            store_eng.dma_start(out=dst, in_=src)

    nc.finalize()
    _cached_nc[key] = nc
    return nc


_IDENT = None


def _run(x_slab, ca_slab, loop_iters=None, variant="full", **run_kwargs):
    """x_slab (M, N*L) f32, ca_slab (M, N) f32 -> (M, ONC) f32."""
    from concourse.bass_utils import run_bass_kernel_spmd

    global _IDENT
    if _IDENT is None:
        _IDENT = np.eye(R, dtype=np.float32)

    nc = _build_nc(loop_iters, variant)
    in_maps = []
    for c in range(NCORES):
        in_maps.append(
            {
                "x": np.ascontiguousarray(x_slab[c * R : (c + 1) * R]),
                "ca": np.ascontiguousarray(ca_slab[c * R : (c + 1) * R]),
                "ident": _IDENT,
            }
        )
    res = run_bass_kernel_spmd(nc, in_maps, core_ids=list(range(NCORES)), **run_kwargs)
    out = np.concatenate(
        [np.asarray(res.results[c]["out"]) for c in range(NCORES)], axis=0
    )
    return out, res


def kernel(x, ca):
    x = np.ascontiguousarray(np.asarray(x, dtype=np.float32).reshape(M, N * L))
    ca = np.ascontiguousarray(np.asarray(ca, dtype=np.float32).reshape(M, N))
    out, _ = _run(x, ca)
    return out.reshape(1, M, ONC, 1)



# revision 41
# speedup vs baseline: 1.0203x; 1.0042x over previous
"""CASSI shear kernel for Trainium2 (Bass/Tile), 8-core SPMD.

Computes, for full inputs x (1, 1024, 1024, 31) and ca (1, 1024, 1024, 1):
    y1[m, n, l] = x[m, n, l] * ca[m, n]
    out[m, j]   = sum_{n+l=j} y1[m, n, l]       (j in [0, 1054))
returning (1, 1024, 1054, 1) float32.

Sharding: rows m across 8 cores (128 rows/core = one full SBUF partition
block). Per core, free dim holds the (n, l) plane contiguously (n-major,
matching HBM layout so DMA loads are fully contiguous per partition).

The kernel is HBM-bound: 16.25 MB of x per core per pass = ~45 us at the
358 GB/s per-core HBM limit. Everything else is sized to hide under that:

  - DMA (SWDGE): x is loaded in 8 chunks of 128 n-columns, cast f32->bf16
    in the DMA datapath (halves SBUF footprint, enables 2x DVE + full-rate
    PE). All 8 chunk tiles stay resident so DMA never stalls on compute.
  - Vector engine: y1 = x * cab in place, bf16 2x_1P mode (~2.2 us/chunk).
    cab is ca broadcast 31x along l, materialized once in the preamble by
    the (otherwise idle) scalar engine, per chunk slice so chunk 0 never
    waits on the whole build.
  - Tensor engine: the 31-way shear scatter-add as bf16 identity-weight
    matmuls accumulating into PSUM. One matmul covers ALL 31 l values over
    a 16-wide n window: out free dims [w=16, g=31] with psum column
    t = t0 + n + k (overlapping within the op is fine -- PSUM accumulation
    is in-memory per element), rhs free dims [w, g] = y1[t0-l0+n, l0+k],
    which is one FLAT CONTIGUOUS bf16 stream. Dim order matters: putting
    the stride-31 n dim innermost instead ran the PE ~4x slower (strided
    rhs fetch). Windows split at PSUM bank boundaries (per-l fallback at
    the two crossings). PSUM has_written is reset by marking the first
    matmul touching each bank start=True (clears the whole bank) -- no
    zero-weight reset matmuls needed.
  - Scalar engine: evacuates each PSUM bank to SBUF as soon as its last
    contribution lands (bank 0 halfway through, banks 1-2 at the end), so
    only the last chunk's compute + one small copy + store sit after the
    final DMA.

The benchmark loop (loop_iters=N) wraps the body in For_i, whose back
edge runs an InstAllEngineBarrier costing ~5-10 us; "full@uN" unrolls N
bodies per back-edge to amortize it (u1/u2/u4/u8/u16/u32 measured
58.3/53.5/51.2/50.1/50.0/50.0 us with the staged-input timing harness
in test.py; u20 edges out u16 by ~0.25 us, and with @bs u40 edges out
u20 by another ~0.2 us over 14 matched rounds; plain u32 had looked
flat). "@bs" batches the two per-body output stores into one full-row
1054-col DMA -- better HBM write locality, another ~0.25 us. The
benched champion is full@bs@u40 at ~49.4-49.5 us.

Measured steady-state decomposition at u16 (per iteration, per core):
load stream 46.5 us (16.25 MB = 350 GB/s, ~98% of the 358 GB/s
HBM-per-NC limit), + mul 0.3, + PE/evac 0.3, + output stores 2.9
(1.4 us of write bytes + ~1.5 us HBM read/write-turnaround tax).

Dead ends measured (clean instrument, all within noise or worse):
"@hw" HWDGE f32 loads + DVE fused cast-mul (dodges the DVE-2port/SWDGE
descriptor-starvation trap -- but that trap only costs ~0.3 us here),
"@2q" alternating loads on both HWDGE rings, "@dgN" grouped bigger
DMAs, "@bs" single batched store, "@ds" stores deferred to the block
edge, "@gs" stores via SWDGE, "@s" staggered-reset stages (much
slower -- stage barriers break chunk pipelining), "@h" PE branch
prefetch hint, "@nc" no-carry evacuation (extra PE pieces cost more
than the DVE carry merges they remove), "@ck256"/"@ck512" coarser
chunking (neutral on the pure DMA stream, worse end-to-end: +1.2 us
at ck256, +4 us at ck512 where 2 xp bufs stall the pipeline). Only
"@bs" (store batching) beat the turnaround tax, and only by ~0.25 us
of the ~1.5. The store turnaround tax is
proportional to write bytes and survives every relocation of the
stores, so ~49.3 us (load stream + store bytes + barrier/16) is the
practical floor; the kernel sits ~0.7 us above it.
"""

import sys

import numpy as np

if "/opt/trn_rl_repo" not in sys.path:
    sys.path.insert(0, "/opt/trn_rl_repo")

M, N, L = 1024, 1024, 31
ONC = N + L - 1  # 1054
NCORES = 8
R = M // NCORES  # 128 rows per core
CHUNK = 128
BANK = 512  # PSUM bank size in fp32 elements

_cached_nc = {}


def _shear_pieces(chunk, gmax=31, carry=False):
    """All shear matmuls as {chunk_idx: [(l0, g, t0, w, start, stop)]}.

    Each matmul handles a group of g l-values {l0..l0+g-1} over the
    chunk's n-window: out free dims [g, w] with psum column t = t0 + k + n
    (overlapping within the op is fine -- PSUM accumulation is in-memory
    per element), rhs free dims [g, w] reading y1[(t0 - l0) + n, l0 + k].

    Windows split so each piece stays inside one PSUM bank. start=True
    marks the first matmul touching each bank (clears has_written for the
    whole bank -> accumulator resets with zero extra instructions);
    stop=True marks the last, gating that bank's evacuation.
    """
    if isinstance(chunk, int):
        chunks = [(i * chunk, chunk) for i in range(N // chunk)]
    else:
        chunks = chunk
    pieces = []
    for i, (n0, cw) in enumerate(chunks):
        for l0 in range(0, L, gmax):
            g = min(gmax, L - l0)
            wmax = BANK // gmax  # keep out free size within one bank
            t0 = n0 + l0
            remaining = cw
            while remaining > 0:
                bank_end = (t0 // BANK + 1) * BANK
                w = min(remaining, wmax, bank_end - t0 - (g - 1))
                if w < 1:
                    if carry:
                        # group span straddles the bank boundary: route
                        # the whole straddle rectangle (all g l's, the
                        # n's whose span crosses) into the carry bank as
                        # ONE matmul; merged into the output during
                        # evacuation. Carry col = CARRY_OFF[be] + (out
                        # col - (be - (g-1) - 1))... here simply
                        # cdst = carry base + (t0 - (be - 30)).
                        # out free size (wc*g) must stay <= 512 (fp32
                        # PSUM ISA limit), so split the straddle region
                        # into wmax-wide windows like the main pieces
                        wc = min(remaining, bank_end - t0, wmax)
                        cbase = 0 if bank_end == BANK else 64
                        cdst = cbase + (t0 - (bank_end - 30))
                        assert 0 <= cdst and cdst + wc - 1 + (g - 1) < cbase + 60
                        pieces.append(
                            [i, l0, g, t0, wc, False, False, cdst]
                        )
                        t0 += wc
                        remaining -= wc
                        continue
                    # no-carry fallback: emit the rest of this window
                    # per-l (small free dims)
                    for k in range(g):
                        aa, rem2 = t0 + k, remaining
                        while rem2 > 0:
                            be = (aa // BANK + 1) * BANK
                            w2 = min(rem2, be - aa)
                            pieces.append(
                                [i, l0 + k, 1, aa, w2, False, False, None]
                            )
                            aa += w2
                            rem2 -= w2
                    break
                pieces.append([i, l0, g, t0, w, False, False, None])
                t0 += w
                remaining -= w
    first_by_bank, last_by_bank = {}, {}
    for idx, (_, _, g, t0, w, _, _, cdst) in enumerate(pieces):
        # a piece touches banks floor(t0/BANK) .. floor((t0+g-1+w-1)/BANK);
        # by construction it stays in one bank ("carry" = the carry bank)
        b = "carry" if cdst is not None else t0 // BANK
        first_by_bank.setdefault(b, idx)
        last_by_bank[b] = idx
    for idx in first_by_bank.values():
        pieces[idx][5] = True
    for idx in last_by_bank.values():
        pieces[idx][6] = True
    by_chunk = {}
    for i, l0, g, t0, w, start, stop, cdst in pieces:
        by_chunk.setdefault(i, []).append((l0, g, t0, w, start, stop, cdst))
    # which banks see their final write in chunk i (drives evacuation)
    done_banks = {}
    for b, idx in last_by_bank.items():
        if b != "carry":
            done_banks.setdefault(pieces[idx][0], []).append(b)
    return by_chunk, done_banks


def _build_nc(loop_iters=None, variant="full"):
    """Build the per-core Bass program. loop_iters wraps the body in an
    on-device For_i repeating the computation (for benchmarking); None
    runs it once. variant: "full", or "+"-joined flags out of
    {dma, mul, pe} with optional "@u<unroll>" suffix."""
    key = (loop_iters, variant)
    if key in _cached_nc:
        return _cached_nc[key]

    import concourse.bass as bass
    import concourse.mybir as mybir
    from concourse import bacc
    from concourse.tile import TileContext

    f32 = mybir.dt.float32
    bf16 = mybir.dt.bfloat16
    nc = bacc.Bacc("TRN2")

    xin = nc.dram_tensor("x", (R, N * L), f32, kind="ExternalInput")
    cain = nc.dram_tensor("ca", (R, N), f32, kind="ExternalInput")
    identin = nc.dram_tensor("ident", (R, R), f32, kind="ExternalInput")
    outd = nc.dram_tensor("out", (R, ONC), f32, kind="ExternalOutput")

    toks = variant.split("@")
    vspec = toks[0]
    unroll, gmax, staggered, hint, tapered, use_carry = 1, 31, False, False, False, True
    use_hw, dma_group, no_store, two_q = False, 1, False, False
    batch_store, gp_store, defer_store, xp_bufs = False, False, False, None
    chunk_w = CHUNK
    for t in toks[1:]:
        if t == "s":
            staggered = True
        elif t == "h":
            hint = True
        elif t == "t":
            tapered = True
        elif t == "c":
            use_carry = True
        elif t == "nc":
            use_carry = False
        elif t == "hw":
            use_hw = True
        elif t == "2q":
            two_q = True
        elif t == "ns":
            no_store = True
        elif t == "bs":
            batch_store = True
        elif t == "gs":
            gp_store = True
        elif t == "ds":
            defer_store = True
        elif t.startswith("xb"):
            xp_bufs = int(t[2:])
        elif t.startswith("ck"):
            chunk_w = int(t[2:])
        elif t.startswith("dg"):
            dma_group = int(t[2:])
        elif t.startswith("u"):
            unroll = int(t[1:])
        elif t.startswith("g"):
            gmax = int(t[1:])
    # the carry-merge geometry holds for chunk widths where the two bank
    # straddles (cols 482.., 994..) land in chunks nchunks//2-1 and
    # nchunks-1 with the same carry-bank offsets: 128/256/512
    if tapered or gmax != 31 or chunk_w not in (128, 256, 512):
        use_carry = False
    if vspec == "full":
        flags = {"dma", "mul", "pe"}
    else:
        flags = set(vspec.split("+"))
    if tapered:
        # split the last 128-col chunk in two: halves the serial tail
        # (mul + shear of the final chunk) behind the last DMA
        chunks = [(i * CHUNK, CHUNK) for i in range(N // CHUNK - 1)]
        h = CHUNK // 2
        chunks += [(N - CHUNK, h), (N - h, h)]
    else:
        chunks = [(i * chunk_w, chunk_w) for i in range(N // chunk_w)]
    nchunks = len(chunks)
    by_chunk, done_banks = _shear_pieces(chunks, gmax, carry=use_carry)
    if use_carry:
        # hardcoded merge geometry below assumes this piece layout
        assert not tapered and gmax == 31
        carry_ia = nchunks // 2 - 1  # chunk containing the col-482 straddle
        carry_ib = nchunks - 1  # chunk containing the col-994 straddle
        assert sorted(done_banks.get(carry_ia, [])) == [0]
        assert sorted(done_banks.get(carry_ib, [])) == [1]

    # hw mode: x loads as f32 on HWDGE (sync engine). SWDGE cast-DMAs are
    # starved whenever DVE runs a two-read-port op (the tensor_tensor mul
    # holds the DVE/GpSimd shared SBUF port pair, blocking Q7 descriptor
    # generation); HWDGE has no SBUF descriptor rings, so it is immune.
    # The f32->bf16 cast folds into the multiply (DVE converts on write),
    # and ca is read via a stride-0 broadcast AP, so the cab materialization
    # disappears too. Stores ride the other HWDGE ring (scalar/ACT).
    if use_hw:
        assert nchunks % dma_group == 0
        store_eng = nc.scalar
    else:
        dma_group = 1
        store_eng = nc.sync
    if gp_store:
        store_eng = nc.gpsimd

    acc_bufs = (unroll + 1) if defer_store else 2
    with TileContext(nc) as tc:
        with (
            tc.tile_pool(name="xp", bufs=xp_bufs or ((4 if dma_group <= 2 else 2) if use_hw else nchunks)) as xp,
            tc.tile_pool(name="yp", bufs=4) as yp,
            tc.tile_pool(name="cp", bufs=1) as cp,
            tc.tile_pool(name="accp", bufs=acc_bufs) as accp,
            tc.tile_pool(name="pp", bufs=2, space="PSUM") as pp,
        ):
            ca_t = cp.tile([R, N], f32)
            nc.sync.dma_start(out=ca_t[:], in_=cain[:])
            # bf16 identity for the shear matmuls (cast during DMA)
            idb = cp.tile([R, R], bf16, tag="idb")
            nc.gpsimd.dma_start(out=idb[:], in_=identin[:])

            cab = None
            if not use_hw:
                # cab[m, n*L + l] = ca[m, n] as bf16: built once, per chunk
                # slice, on the scalar engine (idle during the main loop)
                cab = cp.tile([R, N * L], bf16, tag="cab")
                cab3 = cab[:].rearrange("p (n l) -> p n l", l=L)
                for n0, cw in chunks:
                    src = (
                        ca_t[:, n0 : n0 + cw]
                        .unsqueeze(2)
                        .broadcast_to([R, cw, L])
                    )
                    nc.scalar.copy(cab3[:, n0 : n0 + cw], src)

            def body(marks=(), pending=None):
                def store(dst, src):
                    if no_store:
                        return
                    if pending is None:
                        store_eng.dma_start(out=dst, in_=src)
                    else:
                        pending.append((dst, src))

                pacc = pp.tile([R, ONC], f32, tag="pacc")
                pc = None
                if use_carry:
                    pc = pp.tile([R, 128], f32, tag="carry")
                xts = {}
                for i, (n0, cw) in enumerate(chunks):
                    if i in marks:
                        tc.stage_boundary()
                    if use_hw:
                        if i % dma_group == 0:
                            gw = sum(c for _, c in chunks[i : i + dma_group])
                            gt = xp.tile([R, gw * L], f32, tag="xgrp")
                            if "dma" in flags:
                                # alternate loads across the two HWDGE
                                # rings (qSP / qAct) when two_q
                                ldq = (
                                    nc.scalar
                                    if two_q and (i // dma_group) % 2
                                    else nc.sync
                                )
                                ldq.dma_start(
                                    out=gt[:],
                                    in_=xin[:, n0 * L : n0 * L + gw * L],
                                )
                            for j, (m0, mw) in enumerate(
                                chunks[i : i + dma_group]
                            ):
                                off = (m0 - n0) * L
                                xts[i + j] = gt[:, off : off + mw * L]
                        yv = None
                        if "mul" in flags:
                            yt = yp.tile([R, cw * L], bf16, tag="ychunk")
                            src3 = xts[i].rearrange(
                                "p (n l) -> p n l", l=L
                            )
                            cb3 = (
                                ca_t[:, n0 : n0 + cw]
                                .unsqueeze(2)
                                .broadcast_to([R, cw, L])
                            )
                            y3 = yt[:].rearrange("p (n l) -> p n l", l=L)
                            nc.vector.tensor_tensor(
                                y3, src3, cb3, mybir.AluOpType.mult
                            )
                            yv = yt[:]
                    else:
                        xt = xp.tile([R, cw * L], bf16, tag="xchunk")
                        if "dma" in flags:
                            # f32 -> bf16 cast in the DMA datapath (SWDGE)
                            nc.gpsimd.dma_start(
                                out=xt[:], in_=xin[:, n0 * L : (n0 + cw) * L]
                            )
                        yv = xt[:]
                        if "mul" in flags:
                            # in-place broadcast multiply, bf16 2x_1P (both
                            # operands contiguous step-1 bf16)
                            nc.vector.tensor_tensor(
                                yv,
                                yv,
                                cab[:, n0 * L : (n0 + cw) * L],
                                mybir.AluOpType.mult,
                            )
                    if "pe" in flags and yv is not None:
                        part = [int(yv.ap[0][0]), int(yv.ap[0][1])]
                        for l0, g, t0, w, start, stop, cdst in by_chunk[i]:
                            # out col t = t0 + k + n (overlap inside the
                            # op is fine; PSUM accumulation is in-memory);
                            # rhs elem (n,k) = y1[(t0-l0-n0)+n, l0+k].
                            # Dim order: n outer, l-group inner -- the
                            # innermost run is contiguous in SBUF (the PE
                            # rhs fetch rate collapses on strided inner
                            # reads; at g=31 the whole stream is flat)
                            rhs = bass.AP(
                                yv.tensor,
                                yv.offset + (t0 - l0 - n0) * L + l0,
                                [part, [L, w], [1, g]],
                            )
                            if cdst is None:
                                pv = pacc[:, t0 : t0 + (g - 1) + w]
                            else:
                                pv = pc[:, cdst : cdst + (g - 1) + w]
                            pp0 = [int(pv.ap[0][0]), int(pv.ap[0][1])]
                            dst = bass.AP(
                                pv.tensor, pv.offset, [pp0, [1, w], [1, g]]
                            )
                            nc.tensor.matmul(
                                dst,
                                idb[:],
                                rhs,
                                start=start,
                                stop=stop,
                                skip_group_check=True,
                            )
                        if use_carry:
                            # explicit evacuation with carry merges
                            # (geometry asserted above)
                            add = mybir.AluOpType.add
                            if i == carry_ia:
                                if batch_store:
                                    acc = accp.tile([R, ONC], f32,
                                                    tag="accall")
                                    body.acc = acc
                                    at0 = acc[:, 0:BANK]
                                else:
                                    at0t = accp.tile([R, BANK], f32,
                                                     tag="acc0")
                                    at0 = at0t[:]
                                nc.scalar.copy(at0, pacc[:, 0:BANK])
                                nc.vector.tensor_tensor(
                                    at0[:, 482:512], at0[:, 482:512],
                                    pc[:, 0:30], add,
                                )
                                if not batch_store:
                                    store(outd[:, 0:BANK], at0)
                            elif i == carry_ib:
                                if batch_store:
                                    at1 = body.acc[:, BANK:ONC]
                                else:
                                    at1t = accp.tile([R, ONC - BANK], f32,
                                                     tag="acc1")
                                    at1 = at1t[:]
                                nc.scalar.copy(
                                    at1[:, 0:512], pacc[:, 512:1024]
                                )
                                nc.vector.tensor_tensor(
                                    at1[:, 0:30], at1[:, 0:30],
                                    pc[:, 30:60], add,
                                )
                                nc.vector.tensor_tensor(
                                    at1[:, 482:512], at1[:, 482:512],
                                    pc[:, 64:94], add,
                                )
                                nc.vector.tensor_copy(
                                    at1[:, 512:542], pc[:, 94:124]
                                )
                                if batch_store:
                                    store(outd[:, 0:ONC], body.acc[:])
                                else:
                                    store(outd[:, BANK:ONC], at1)
                            continue
                        # evacuate any PSUM banks whose last contribution
                        # just landed (adjacent banks coalesced); store
                        # them right away
                        bs = sorted(done_banks.get(i, []))
                        while bs:
                            b0 = b1 = bs.pop(0)
                            while bs and bs[0] == b1 + 1:
                                b1 = bs.pop(0)
                            a0 = b0 * BANK
                            a1 = min((b1 + 1) * BANK, ONC)
                            at = accp.tile([R, a1 - a0], f32, tag=f"acc{b0}")
                            nc.scalar.copy(at[:], pacc[:, a0:a1])
                            store(outd[:, a0:a1], at[:])

            if loop_iters is None:
                body()
            else:
                u = max(u for u in (unroll, 1) if loop_iters % u == 0)
                hints = (mybir.EngineType.PE,) if hint else ()
                stag = staggered and (u * nchunks) % 4 == 0
                # staggered_reset needs exactly 4 stages per loop body;
                # spread them evenly over the unrolled copies' chunks
                span = (u * nchunks) // 4
                with tc.For_i(
                    0, loop_iters // u, 1,
                    hint_engines=hints, staggered_reset=stag,
                ):
                    pend = [] if defer_store else None
                    for j in range(u):
                        if stag:
                            marks = {
                                (s * span) - j * nchunks
                                for s in range(1, 4)
                                if 0 <= (s * span) - j * nchunks < nchunks
                            }
                        else:
                            marks = ()
                        body(marks, pend)
                    if pend:
                        # flush all deferred output stores at the block
                        # edge: the write bytes drain while the loop
                        # barrier has the load stream quiesced instead of
                        # displacing read bandwidth mid-iteration
                        for dst, src in pend:
                            store_eng.dma_start(out=dst, in_=src)

    nc.finalize()
    _cached_nc[key] = nc
    return nc


_IDENT = None


def _run(x_slab, ca_slab, loop_iters=None, variant="full", **run_kwargs):
    """x_slab (M, N*L) f32, ca_slab (M, N) f32 -> (M, ONC) f32."""
    from concourse.bass_utils import run_bass_kernel_spmd

    global _IDENT
    if _IDENT is None:
        _IDENT = np.eye(R, dtype=np.float32)

    nc = _build_nc(loop_iters, variant)
    in_maps = []
    for c in range(NCORES):
        in_maps.append(
            {
                "x": np.ascontiguousarray(x_slab[c * R : (c + 1) * R]),
                "ca": np.ascontiguousarray(ca_slab[c * R : (c + 1) * R]),
                "ident": _IDENT,
            }
        )
    res = run_bass_kernel_spmd(nc, in_maps, core_ids=list(range(NCORES)), **run_kwargs)
    out = np.concatenate(
        [np.asarray(res.results[c]["out"]) for c in range(NCORES)], axis=0
    )
    return out, res


def kernel(x, ca):
    x = np.ascontiguousarray(np.asarray(x, dtype=np.float32).reshape(M, N * L))
    ca = np.ascontiguousarray(np.asarray(ca, dtype=np.float32).reshape(M, N))
    out, _ = _run(x, ca)
    return out.reshape(1, M, ONC, 1)

